# revision 38
# baseline (speedup 1.0000x reference)
"""Additive (Bahdanau) attention kernel for Trainium2, 8 NeuronCores.

Math: the reference computes
    s1 = f1 @ w   [N,L];  s2 = f2 @ w   [N,T]
    att = softmax(s2[:,:,None] + s1[:,None,:] + b, axis=2)   [N,T,L]
    f_hat = att @ f1                                          [N,T,D]
Softmax along l is invariant to the per-(n,t) constant s2[n,t]+b, so
    att[n,t,l] = softmax_l(s1[n,l])      (independent of t; f2, b unused)
    f_hat[n,t,:] = p[n,:] @ f1[n]        (independent of t)
Each core handles 8 batches: streams f1 tiles, computes the 512-wide
softmax once per batch, one [1,512]@[512,1024] GEMM per batch (emitted
as M=128 via a free-broadcast stationary column so the PSUM result is
already replicated across the t axis), and broadcasts att across t via
gpsimd partition_broadcast.
"""

import numpy as np

N, L, D, T = 64, 512, 1024, 128
NCORES = 8
NB = N // NCORES  # batches per core
P = 128
LT = L // P       # l-tiles per batch
USE_F32R = True

_CACHE = {}


def _build():
    import concourse.bass as bass
    import concourse.bacc as bacc
    import concourse.tile as tile
    from concourse import mybir
    from concourse.masks import make_identity

    f32 = mybir.dt.float32
    mm_dt = mybir.dt.float32r if USE_F32R else f32
    AX = mybir.AxisListType
    ALU = mybir.AluOpType
    ACTF = mybir.ActivationFunctionType

    nc = bacc.Bacc("TRN2", target_bir_lowering=False, debug=False)
    f1 = nc.dram_tensor("feature_1", [NB, L, D], mm_dt, kind="ExternalInput")
    w = nc.dram_tensor("w", [1, D], f32, kind="ExternalInput")
    fhat = nc.dram_tensor("f_hat", [NB, T, D], f32, kind="ExternalOutput")
    att = nc.dram_tensor("att", [NB, T, L], f32, kind="ExternalOutput")

    from concourse.tile import add_dep_helper

    with tile.TileContext(nc) as tc:
        with (
            tc.tile_pool(name="const", bufs=1) as const,
            tc.tile_pool(name="f1p", bufs=8) as f1p,
            tc.tile_pool(name="scratch", bufs=4) as scratch,
            tc.tile_pool(name="outb", bufs=4) as outb,
            tc.tile_pool(name="small", bufs=4) as small,
            tc.tile_pool(name="ps_small", bufs=2, space="PSUM") as ps_small,
            tc.tile_pool(name="ps_one", bufs=1, space="PSUM") as ps_onep,
            tc.tile_pool(name="ps_att", bufs=1, space="PSUM") as ps_attp,
            tc.tile_pool(name="ps_fbar", bufs=2, space="PSUM") as ps_fbarp,
        ):
            # w broadcast to all 128 partitions in one DMA (stride-0 DRAM src)
            w_bc = const.tile([P, D], f32)
            wap = w[:]
            w_bcast_src = bass.AP(tensor=wap.tensor, offset=wap.offset,
                                  ap=[[0, P]] + list(wap.ap[1:]))
            nc.scalar.dma_start(out=w_bc[:], in_=w_bcast_src)
            ident = const.tile([P, P], f32)
            make_identity(nc, ident[:])
            ones_row = const.tile([1, P], f32)
            nc.vector.memset(ones_row[:], 1.0)

            in_dmas = []
            for n in range(NB):
                # phase 1: one 2MB DMA per batch loads all 4 l-tiles into a
                # [128, LT, D] tile (dst partition p <-> src row lt*128+p);
                # fused mul+reduce -> s1^T columns.  Input DMAs are chained so
                # at most 2 are in flight: the HW otherwise runs every queued
                # transfer concurrently and the oldest (most urgent) one gets
                # starved of bandwidth.
                s1T = small.tile([P, LT], f32, tag="s1T")
                ftall = f1p.tile([P, LT, D], mm_dt, tag="f1t")
                src = f1[n].rearrange("(lt p) d -> p lt d", p=P)
                if n == 0:
                    # batch 0 loads in 4 chunks; chunk 0 runs alone (all other
                    # input DMAs wait on it) so the pipeline primes in ~1.5us
                    # instead of being bandwidth-starved by the flood.
                    for lt in range(LT):
                        d = nc.sync.dma_start(out=ftall[:, lt, :],
                                              in_=src[:, lt, :])
                        in_dmas.append(d)
                        if lt > 0:
                            add_dep_helper(d.ins, in_dmas[0].ins,
                                           reason="protect first chunk")
                else:
                    d = nc.sync.dma_start(out=ftall[:], in_=src)
                    add_dep_helper(d.ins, in_dmas[0].ins,
                                   reason="protect first chunk")
                for lt in range(LT):
                    so = scratch.tile([P, D], f32, tag="ttr")
                    nc.vector.scalar_tensor_tensor(
                        out=so[:], in0=ftall[:, lt, :].bitcast(f32), scalar=1.0,
                        in1=w_bc[:],
                        op0=ALU.mult, op1=ALU.mult,
                        accum_out=s1T[:, lt:lt + 1],
                    )
                # phase 2: per-ltile transpose of s1^T columns straight into a
                # [1, L] psum row, copy once to SBUF, softmax, transpose p back
                ps1 = ps_onep.tile([1, L], f32, tag="ps1")
                for lt in range(LT):
                    nc.tensor.transpose(ps1[:, lt * P:(lt + 1) * P],
                                        s1T[:, lt:lt + 1], ident[:])
                # scores are O(1) (w ~ U(-1/32,1/32), D=1024 -> |s1| <~ 4), so
                # exp never overflows f32 and the max-subtraction can be
                # dropped -- softmax is shift-invariant, result identical.
                # exp reads PSUM directly; normalization by 1/sum is folded
                # into the PSUM->SBUF output copies as a per-partition scale.
                p_nl = small.tile([1, L], mm_dt, tag="pnl")
                ssum = small.tile([1, 1], f32, tag="ssum")
                nc.scalar.activation(out=p_nl[:], in_=ps1[:], func=ACTF.Exp,
                                     bias=0.0, scale=1.0, accum_out=ssum[:])
                rinv = small.tile([1, 1], f32, tag="rinv")
                nc.vector.reciprocal(rinv[:], ssum[:])
                rinv_bc = small.tile([P, 1], f32, tag="rinvbc")
                nc.gpsimd.partition_broadcast(rinv_bc[:], rinv[:])

                # att[n]: broadcast p across the 128 t rows via a K=1
                # ones-column matmul (PSUM gets p replicated on all partitions)
                psa = ps_attp.tile([P, L], f32, tag="psa")
                nc.tensor.matmul(psa[:], ones_row[:], p_nl[:].bitcast(f32),
                                 start=True, stop=True)
                att_sb = outb.tile([P, L], f32, tag="att_sb")
                if n % 2 == 0:
                    nc.scalar.activation(out=att_sb[:], in_=psa[:], func=ACTF.Copy,
                                         bias=0.0, scale=rinv_bc[:])
                else:
                    nc.vector.tensor_scalar_mul(att_sb[:], psa[:], rinv_bc[:])
                nc.sync.dma_start(out=att[n], in_=att_sb[:])

                # p^T columns for the GEMM
                ps_pT = ps_small.tile([P, LT], f32, tag="pspT")
                for lt in range(LT):
                    nc.tensor.transpose(ps_pT[:, lt:lt + 1],
                                        p_nl[:, lt * P:(lt + 1) * P].bitcast(f32),
                                        ident[:1, :1])
                pT = small.tile([P, LT], mm_dt, tag="pT")
                nc.vector.tensor_copy(out=pT[:], in_=ps_pT[:])

                # phase 3: fbar = p @ f1, stationary column broadcast to M=128
                # so PSUM holds fbar replicated across all t partitions.
                psf = ps_fbarp.tile([P, D], f32, tag="psf")
                for lt in range(LT):
                    col = pT[:, lt:lt + 1]
                    lhsT = bass.AP(tensor=col.tensor, offset=col.offset,
                                   ap=[list(col.ap[0]), [0, P]])
                    for h in range(2):
                        nc.tensor.matmul(
                            psf[:, h * 512:(h + 1) * 512],
                            lhsT,
                            ftall[:, lt, h * 512:(h + 1) * 512],
                            start=(lt == 0), stop=(lt == LT - 1),
                        )
                fhat_sb = outb.tile([P, D], f32, tag="fhat_sb")
                if n % 2 == 0:
                    nc.vector.tensor_scalar_mul(fhat_sb[:], psf[:], rinv_bc[:])
                else:
                    nc.scalar.activation(out=fhat_sb[:], in_=psf[:], func=ACTF.Copy,
                                         bias=0.0, scale=rinv_bc[:])
                nc.sync.dma_start(out=fhat[n], in_=fhat_sb[:])

    nc.compile()
    return nc


def _get_nc():
    if "nc" not in _CACHE:
        _CACHE["nc"] = _build()
    return _CACHE["nc"]


def _in_maps(feature_1, w):
    f1 = np.ascontiguousarray(np.asarray(feature_1, dtype=np.float32))
    wv = np.ascontiguousarray(np.asarray(w, dtype=np.float32).reshape(1, D))
    return [{"feature_1": f1[i * NB:(i + 1) * NB], "w": wv} for i in range(NCORES)]


def _run(feature_1, w, trace=False):
    from concourse.bass_utils import run_bass_kernel_spmd
    nc = _get_nc()
    res = run_bass_kernel_spmd(nc, _in_maps(feature_1, w),
                               core_ids=list(range(NCORES)), trace=trace)
    f_hat = np.concatenate([r["f_hat"] for r in res.results], axis=0)
    att = np.concatenate([r["att"] for r in res.results], axis=0)
    return (f_hat, att), res


def kernel(feature_1, feature_2=None, w=None, b=None, **_ignored):
    (f_hat, att), _ = _run(feature_1, w)
    return (f_hat, att)


# revision 39
# speedup vs baseline: 1.1725x; 1.1725x over previous
"""Additive (Bahdanau) attention kernel for Trainium2, 8 NeuronCores.

Math: the reference computes
    s1 = f1 @ w   [N,L];  s2 = f2 @ w   [N,T]
    att = softmax(s2[:,:,None] + s1[:,None,:] + b, axis=2)   [N,T,L]
    f_hat = att @ f1                                          [N,T,D]
Softmax along l is invariant to the per-(n,t) constant s2[n,t]+b, so
    att[n,t,l] = softmax_l(s1[n,l])      (independent of t; f2, b unused)
    f_hat[n,t,:] = p[n,:] @ f1[n]        (independent of t)
Each core handles 8 batches: streams f1 tiles, computes the 512-wide
softmax once per batch, one [1,512]@[512,1024] GEMM per batch (emitted
as M=128 via a free-broadcast stationary column so the PSUM result is
already replicated across the t axis), and broadcasts att across t via
gpsimd partition_broadcast.
"""

import numpy as np

N, L, D, T = 64, 512, 1024, 128
NCORES = 8
NB = N // NCORES  # batches per core
P = 128
LT = L // P       # l-tiles per batch
USE_F32R = True

_CACHE = {}


def _build():
    import concourse.bass as bass
    import concourse.bacc as bacc
    import concourse.tile as tile
    from concourse import mybir
    from concourse.masks import make_identity

    f32 = mybir.dt.float32
    mm_dt = mybir.dt.float32r if USE_F32R else f32
    AX = mybir.AxisListType
    ALU = mybir.AluOpType
    ACTF = mybir.ActivationFunctionType

    nc = bacc.Bacc("TRN2", target_bir_lowering=False, debug=False)
    f1 = nc.dram_tensor("feature_1", [NB, L, D], mm_dt, kind="ExternalInput")
    w = nc.dram_tensor("w", [1, D], f32, kind="ExternalInput")
    fhat = nc.dram_tensor("f_hat", [NB, T, D], f32, kind="ExternalOutput")
    att = nc.dram_tensor("att", [NB, T, L], f32, kind="ExternalOutput")

    from concourse.tile import add_dep_helper

    with tile.TileContext(nc) as tc:
        with (
            tc.tile_pool(name="const", bufs=1) as const,
            tc.tile_pool(name="f1p", bufs=8) as f1p,
            tc.tile_pool(name="scratch", bufs=4) as scratch,
            tc.tile_pool(name="outb", bufs=4) as outb,
            tc.tile_pool(name="small", bufs=4) as small,
            tc.tile_pool(name="ps_small", bufs=2, space="PSUM") as ps_small,
            tc.tile_pool(name="ps_one", bufs=1, space="PSUM") as ps_onep,
            tc.tile_pool(name="ps_att", bufs=1, space="PSUM") as ps_attp,
            tc.tile_pool(name="ps_fbar", bufs=2, space="PSUM") as ps_fbarp,
        ):
            # w broadcast to all 128 partitions in one DMA (stride-0 DRAM src)
            w_bc = const.tile([P, D], f32)
            wap = w[:]
            w_bcast_src = bass.AP(tensor=wap.tensor, offset=wap.offset,
                                  ap=[[0, P]] + list(wap.ap[1:]))
            nc.scalar.dma_start(out=w_bc[:], in_=w_bcast_src)
            ident = const.tile([P, P], f32)
            make_identity(nc, ident[:])
            ones_row = const.tile([1, P], f32)
            nc.vector.memset(ones_row[:], 1.0)

            in_dmas = []
            for n in range(NB):
                # phase 1: one 2MB DMA per batch loads all 4 l-tiles into a
                # [128, LT, D] tile (dst partition p <-> src row lt*128+p);
                # fused mul+reduce -> s1^T columns.  Input DMAs are chained so
                # at most 2 are in flight: the HW otherwise runs every queued
                # transfer concurrently and the oldest (most urgent) one gets
                # starved of bandwidth.
                s1T = small.tile([P, LT], f32, tag="s1T")
                ftall = f1p.tile([P, LT, D], mm_dt, tag="f1t")
                src = f1[n].rearrange("(lt p) d -> p lt d", p=P)
                if n == 0:
                    # batch 0 loads in 4 chunks; chunk 0 runs alone (all other
                    # input DMAs wait on it) so the pipeline primes in ~1.5us
                    # instead of being bandwidth-starved by the flood.
                    for lt in range(LT):
                        d = nc.sync.dma_start(out=ftall[:, lt, :],
                                              in_=src[:, lt, :])
                        in_dmas.append(d)
                        if lt > 0:
                            add_dep_helper(d.ins, in_dmas[0].ins,
                                           reason="protect first chunk")
                else:
                    d = nc.sync.dma_start(out=ftall[:], in_=src)
                    add_dep_helper(d.ins, in_dmas[0].ins,
                                   reason="protect first chunk")
                for lt in range(LT):
                    so = scratch.tile([P, D], f32, tag="ttr")
                    nc.vector.scalar_tensor_tensor(
                        out=so[:], in0=ftall[:, lt, :].bitcast(f32), scalar=1.0,
                        in1=w_bc[:],
                        op0=ALU.mult, op1=ALU.mult,
                        accum_out=s1T[:, lt:lt + 1],
                    )
                # phase 2: per-ltile transpose of s1^T columns straight into a
                # [1, L] psum row, copy once to SBUF, softmax, transpose p back
                ps1 = ps_onep.tile([1, L], f32, tag="ps1")
                for lt in range(LT):
                    nc.tensor.transpose(ps1[:, lt * P:(lt + 1) * P],
                                        s1T[:, lt:lt + 1], ident[:])
                # scores are O(1) (w ~ U(-1/32,1/32), D=1024 -> |s1| <~ 4), so
                # exp never overflows f32 and the max-subtraction can be
                # dropped -- softmax is shift-invariant, result identical.
                # exp reads PSUM directly; normalization by 1/sum is folded
                # into the PSUM->SBUF output copies as a per-partition scale.
                p_nl = small.tile([1, L], mm_dt, tag="pnl")
                ssum = small.tile([1, 1], f32, tag="ssum")
                nc.scalar.activation(out=p_nl[:], in_=ps1[:], func=ACTF.Exp,
                                     bias=0.0, scale=1.0, accum_out=ssum[:])
                rinv = small.tile([1, 1], f32, tag="rinv")
                nc.vector.reciprocal(rinv[:], ssum[:])
                rinv_bc = small.tile([P, 1], f32, tag="rinvbc")
                nc.gpsimd.partition_broadcast(rinv_bc[:], rinv[:])

                # att[n]: broadcast p across the 128 t rows via a K=1
                # ones-column matmul (PSUM gets p replicated on all partitions)
                psa = ps_attp.tile([P, L], f32, tag="psa")
                nc.tensor.matmul(psa[:], ones_row[:], p_nl[:].bitcast(f32),
                                 start=True, stop=True)
                att_sb = outb.tile([P, L], f32, tag="att_sb")
                if n % 2 == 0:
                    nc.scalar.activation(out=att_sb[:], in_=psa[:], func=ACTF.Copy,
                                         bias=0.0, scale=rinv_bc[:])
                else:
                    nc.vector.tensor_scalar_mul(att_sb[:], psa[:], rinv_bc[:])
                nc.scalar.dma_start(out=att[n], in_=att_sb[:])

                # p^T columns for the GEMM
                ps_pT = ps_small.tile([P, LT], f32, tag="pspT")
                for lt in range(LT):
                    nc.tensor.transpose(ps_pT[:, lt:lt + 1],
                                        p_nl[:, lt * P:(lt + 1) * P].bitcast(f32),
                                        ident[:1, :1])
                pT = small.tile([P, LT], mm_dt, tag="pT")
                nc.vector.tensor_copy(out=pT[:], in_=ps_pT[:])

                # phase 3: fbar = p @ f1, stationary column broadcast to M=128
                # so PSUM holds fbar replicated across all t partitions.
                psf = ps_fbarp.tile([P, D], f32, tag="psf")
                for lt in range(LT):
                    col = pT[:, lt:lt + 1]
                    lhsT = bass.AP(tensor=col.tensor, offset=col.offset,
                                   ap=[list(col.ap[0]), [0, P]])
                    for h in range(2):
                        nc.tensor.matmul(
                            psf[:, h * 512:(h + 1) * 512],
                            lhsT,
                            ftall[:, lt, h * 512:(h + 1) * 512],
                            start=(lt == 0), stop=(lt == LT - 1),
                        )
                fhat_sb = outb.tile([P, D], f32, tag="fhat_sb")
                if n % 2 == 0:
                    nc.vector.tensor_scalar_mul(fhat_sb[:], psf[:], rinv_bc[:])
                else:
                    nc.scalar.activation(out=fhat_sb[:], in_=psf[:], func=ACTF.Copy,
                                         bias=0.0, scale=rinv_bc[:])
                nc.scalar.dma_start(out=fhat[n], in_=fhat_sb[:])

    nc.compile()
    return nc


def _get_nc():
    if "nc" not in _CACHE:
        _CACHE["nc"] = _build()
    return _CACHE["nc"]


def _in_maps(feature_1, w):
    f1 = np.ascontiguousarray(np.asarray(feature_1, dtype=np.float32))
    wv = np.ascontiguousarray(np.asarray(w, dtype=np.float32).reshape(1, D))
    return [{"feature_1": f1[i * NB:(i + 1) * NB], "w": wv} for i in range(NCORES)]


def _run(feature_1, w, trace=False):
    from concourse.bass_utils import run_bass_kernel_spmd
    nc = _get_nc()
    res = run_bass_kernel_spmd(nc, _in_maps(feature_1, w),
                               core_ids=list(range(NCORES)), trace=trace)
    f_hat = np.concatenate([r["f_hat"] for r in res.results], axis=0)
    att = np.concatenate([r["att"] for r in res.results], axis=0)
    return (f_hat, att), res


def kernel(feature_1, feature_2=None, w=None, b=None, **_ignored):
    (f_hat, att), _ = _run(feature_1, w)
    return (f_hat, att)


# revision 41
# speedup vs baseline: 1.3073x; 1.1150x over previous
"""Additive (Bahdanau) attention kernel for Trainium2, 8 NeuronCores.

Math: the reference computes
    s1 = f1 @ w   [N,L];  s2 = f2 @ w   [N,T]
    att = softmax(s2[:,:,None] + s1[:,None,:] + b, axis=2)   [N,T,L]
    f_hat = att @ f1                                          [N,T,D]
Softmax along l is invariant to the per-(n,t) constant s2[n,t]+b, so
    att[n,t,l] = softmax_l(s1[n,l])      (independent of t; f2, b unused)
    f_hat[n,t,:] = p[n,:] @ f1[n]        (independent of t)
Each core handles 8 batches: streams f1 tiles, computes the 512-wide
softmax once per batch, one [1,512]@[512,1024] GEMM per batch (emitted
as M=128 via a free-broadcast stationary column so the PSUM result is
already replicated across the t axis), and broadcasts att across t via
gpsimd partition_broadcast.
"""

import numpy as np

N, L, D, T = 64, 512, 1024, 128
NCORES = 8
NB = N // NCORES  # batches per core
P = 128
LT = L // P       # l-tiles per batch
USE_F32R = True

_CACHE = {}


def _build():
    import concourse.bass as bass
    import concourse.bacc as bacc
    import concourse.tile as tile
    from concourse import mybir
    from concourse.masks import make_identity

    f32 = mybir.dt.float32
    mm_dt = mybir.dt.float32r if USE_F32R else f32
    AX = mybir.AxisListType
    ALU = mybir.AluOpType
    ACTF = mybir.ActivationFunctionType

    nc = bacc.Bacc("TRN2", target_bir_lowering=False, debug=False)
    f1 = nc.dram_tensor("feature_1", [NB, L, D], mm_dt, kind="ExternalInput")
    w = nc.dram_tensor("w", [1, D], f32, kind="ExternalInput")
    fhat = nc.dram_tensor("f_hat", [NB, T, D], f32, kind="ExternalOutput")
    att = nc.dram_tensor("att", [NB, T, L], f32, kind="ExternalOutput")

    from concourse.tile import add_dep_helper

    with tile.TileContext(nc) as tc:
        with (
            tc.tile_pool(name="const", bufs=1) as const,
            tc.tile_pool(name="f1p", bufs=8) as f1p,
            tc.tile_pool(name="scratch", bufs=4) as scratch,
            tc.tile_pool(name="outb", bufs=4) as outb,
            tc.tile_pool(name="small", bufs=4) as small,
            tc.tile_pool(name="ps_small", bufs=2, space="PSUM") as ps_small,
            tc.tile_pool(name="ps_one", bufs=1, space="PSUM") as ps_onep,
            tc.tile_pool(name="ps_att", bufs=1, space="PSUM") as ps_attp,
            tc.tile_pool(name="ps_fbar", bufs=2, space="PSUM") as ps_fbarp,
        ):
            # w broadcast to all 128 partitions in one DMA (stride-0 DRAM src)
            w_bc = const.tile([P, D], f32)
            wap = w[:]
            w_bcast_src = bass.AP(tensor=wap.tensor, offset=wap.offset,
                                  ap=[[0, P]] + list(wap.ap[1:]))
            nc.scalar.dma_start(out=w_bc[:], in_=w_bcast_src)
            ident = const.tile([P, P], f32)
            make_identity(nc, ident[:])
            ones_row = const.tile([1, P], f32)
            nc.vector.memset(ones_row[:], 1.0)

            def emit_outputs(n, ftall, p_nl, rinv_bc):
                # att[n]: broadcast p across the 128 t rows via a K=1
                # ones-column matmul (PSUM gets p replicated on all partitions)
                psa = ps_attp.tile([P, L], f32, tag="psa")
                nc.tensor.matmul(psa[:], ones_row[:], p_nl[:].bitcast(f32),
                                 start=True, stop=True)
                att_sb = outb.tile([P, L], f32, tag="att_sb")
                if n % 2 == 0:
                    nc.scalar.activation(out=att_sb[:], in_=psa[:], func=ACTF.Copy,
                                         bias=0.0, scale=rinv_bc[:])
                else:
                    nc.vector.tensor_scalar_mul(att_sb[:], psa[:], rinv_bc[:])
                nc.scalar.dma_start(out=att[n], in_=att_sb[:])

                # p^T columns for the GEMM
                ps_pT = ps_small.tile([P, LT], f32, tag="pspT")
                for lt in range(LT):
                    nc.tensor.transpose(ps_pT[:, lt:lt + 1],
                                        p_nl[:, lt * P:(lt + 1) * P].bitcast(f32),
                                        ident[:1, :1])
                pT = small.tile([P, LT], mm_dt, tag="pT")
                nc.vector.tensor_copy(out=pT[:], in_=ps_pT[:])

                # fbar = p @ f1, stationary column broadcast to M=128 so PSUM
                # holds fbar replicated across all t partitions.
                psf = ps_fbarp.tile([P, D], f32, tag="psf")
                for lt in range(LT):
                    col = pT[:, lt:lt + 1]
                    lhsT = bass.AP(tensor=col.tensor, offset=col.offset,
                                   ap=[list(col.ap[0]), [0, P]])
                    for h in range(2):
                        nc.tensor.matmul(
                            psf[:, h * 512:(h + 1) * 512],
                            lhsT,
                            ftall[:, lt, h * 512:(h + 1) * 512],
                            start=(lt == 0), stop=(lt == LT - 1),
                        )
                fhat_sb = outb.tile([P, D], f32, tag="fhat_sb")
                if n % 2 == 0:
                    nc.vector.tensor_scalar_mul(fhat_sb[:], psf[:], rinv_bc[:])
                else:
                    nc.scalar.activation(out=fhat_sb[:], in_=psf[:], func=ACTF.Copy,
                                         bias=0.0, scale=rinv_bc[:])
                nc.scalar.dma_start(out=fhat[n], in_=fhat_sb[:])

            in_dmas = []
            pend = []
            for n in range(NB):
                # phase 1: one 2MB DMA per batch loads all 4 l-tiles into a
                # [128, LT, D] tile (dst partition p <-> src row lt*128+p);
                # fused mul+reduce -> s1^T columns.  Input DMAs are chained so
                # at most 2 are in flight: the HW otherwise runs every queued
                # transfer concurrently and the oldest (most urgent) one gets
                # starved of bandwidth.
                s1T = small.tile([P, LT], f32, tag="s1T")
                ftall = f1p.tile([P, LT, D], mm_dt, tag="f1t")
                src = f1[n].rearrange("(lt p) d -> p lt d", p=P)
                if n == 0:
                    # batch 0 loads in 4 chunks; chunk 0 runs alone (all other
                    # input DMAs wait on it) so the pipeline primes in ~1.5us
                    # instead of being bandwidth-starved by the flood.
                    for lt in range(LT):
                        d = nc.sync.dma_start(out=ftall[:, lt, :],
                                              in_=src[:, lt, :])
                        in_dmas.append(d)
                        if lt > 0:
                            add_dep_helper(d.ins, in_dmas[0].ins,
                                           reason="protect first chunk")
                else:
                    d = nc.sync.dma_start(out=ftall[:], in_=src)
                    add_dep_helper(d.ins, in_dmas[0].ins,
                                   reason="protect first chunk")
                for lt in range(LT):
                    so = scratch.tile([P, D], f32, tag="ttr")
                    nc.vector.scalar_tensor_tensor(
                        out=so[:], in0=ftall[:, lt, :].bitcast(f32), scalar=1.0,
                        in1=w_bc[:],
                        op0=ALU.mult, op1=ALU.mult,
                        accum_out=s1T[:, lt:lt + 1],
                    )
                # phase 2: per-ltile transpose of s1^T columns straight into a
                # [1, L] psum row, copy once to SBUF, softmax, transpose p back
                ps1 = ps_onep.tile([1, L], f32, tag="ps1")
                for lt in range(LT):
                    nc.tensor.transpose(ps1[:, lt * P:(lt + 1) * P],
                                        s1T[:, lt:lt + 1], ident[:])
                # scores are O(1) (w ~ U(-1/32,1/32), D=1024 -> |s1| <~ 4), so
                # exp never overflows f32 and the max-subtraction can be
                # dropped -- softmax is shift-invariant, result identical.
                # exp reads PSUM directly; normalization by 1/sum is folded
                # into the PSUM->SBUF output copies as a per-partition scale.
                p_nl = small.tile([1, L], mm_dt, tag="pnl")
                ssum = small.tile([1, 1], f32, tag="ssum")
                nc.scalar.activation(out=p_nl[:], in_=ps1[:], func=ACTF.Exp,
                                     bias=0.0, scale=1.0, accum_out=ssum[:])
                rinv = small.tile([1, 1], f32, tag="rinv")
                nc.vector.reciprocal(rinv[:], ssum[:])
                rinv_bc = small.tile([P, 1], f32, tag="rinvbc")
                nc.gpsimd.partition_broadcast(rinv_bc[:], rinv[:])

                # phase 3 (outputs) is emitted one batch LATE so that batch
                # n+1's small s1-transposes sit ahead of batch n's bulk matmul
                # work in the serial PE instruction stream -- otherwise the
                # softmax of each batch stalls behind the previous batch's
                # GEMMs, which stretches the kernel tail.
                pend.append((n, ftall, p_nl, rinv_bc))
                if len(pend) > 1:
                    emit_outputs(*pend.pop(0))
            emit_outputs(*pend.pop(0))

    nc.compile()
    return nc


def _get_nc():
    if "nc" not in _CACHE:
        _CACHE["nc"] = _build()
    return _CACHE["nc"]


def _in_maps(feature_1, w):
    f1 = np.ascontiguousarray(np.asarray(feature_1, dtype=np.float32))
    wv = np.ascontiguousarray(np.asarray(w, dtype=np.float32).reshape(1, D))
    return [{"feature_1": f1[i * NB:(i + 1) * NB], "w": wv} for i in range(NCORES)]


def _run(feature_1, w, trace=False):
    from concourse.bass_utils import run_bass_kernel_spmd
    nc = _get_nc()
    res = run_bass_kernel_spmd(nc, _in_maps(feature_1, w),
                               core_ids=list(range(NCORES)), trace=trace)
    f_hat = np.concatenate([r["f_hat"] for r in res.results], axis=0)
    att = np.concatenate([r["att"] for r in res.results], axis=0)
    return (f_hat, att), res


def kernel(feature_1, feature_2=None, w=None, b=None, **_ignored):
    (f_hat, att), _ = _run(feature_1, w)
    return (f_hat, att)
